# revision 9
# baseline (speedup 1.0000x reference)
"""Trainium2 Bass kernel for the canonical Lp-ECE KDE calibration loss.

Reference computation (see problem statement):
    probs = softmax(input, axis=1)[:, :, ::8, ::8]       -> f [N=8192, C=19]
    y     = argmax(target, axis=1)[:, ::8, ::8]          -> [N]
    alphas = f/0.02 + 1
    log_kern[i,j] = log(f[i]) . (alphas[j]-1) - log_beta[j]   (diag = -inf)
    kern = exp(log_kern);  ratio = (kern @ onehot(y)) / rowsum(kern)
    loss = mean_i sum_c (ratio - f)^2

The O(N^2) part (two GEMMs + 67M exps) runs on 8 NeuronCores, row-sharded:
core k owns rows i in [k*1024, (k+1)*1024).  The j (kernel-center) axis is
*rotated* per core by k*1024 so the self-interaction diagonal always lands
at jlocal == ilocal in [0, 1024) -- a single compiled program (SPMD) can
then mask the diagonal at compile-time-known positions.

Device pipeline per core (ACT-engine bound: ~67M exps / 8 cores):
  GEMM1 (PE):  lognumT[j,i] = sum_K stat[K,j] * mov[K,i]   in bf16 hi/lo
               split arithmetic (K=59: b_hi.a_hi + b_hi.a_lo + b_lo.a_hi
               + (-log_beta) hi/lo rows vs ones), 2x row-group packed.
  diag  (DVE): add -1e30 at diagonal positions of the 8 overlap tiles.
  exp   (ACT): kernT = exp(psum) -> bf16 sbuf, [128,1024] per instruction.
  GEMM2 (PE):  kern_yT[c,i] += y1[j,c] * kernT[j,i], accumulated in psum
               over all 64 j-tiles, 4x column-group packed (M=20<=32).
  epilogue:    combine col groups (PE), den=ones row, clip+reciprocal,
               ratio, (ratio-f)^2, reduce -> per-core partial loss sum.
"""

import numpy as np
import ml_dtypes
from scipy.special import gammaln

import concourse.bass as bass
import concourse.bacc as bacc
import concourse.tile as tile
from concourse import mybir
from concourse.bass_utils import run_bass_kernel_spmd

BF16 = mybir.dt.bfloat16
F32 = mybir.dt.float32
F32R = mybir.dt.float32r
NPBF16 = ml_dtypes.bfloat16

N = 8192          # total pixels after downsampling: 2*64*64
C = 19            # classes
C1 = C + 1        # classes + ones column (row-sum)
NCORES = 8
R = N // NCORES   # rows per core = 1024
KROWS = 59        # contraction rows: 19*3 hi/lo terms + 2 logbeta rows
NT = N // 128     # j tiles = 64
BW = np.float32(0.02)
DF = 8
BIGNEG = -1.0e30


def _build_nc():
    nc = bacc.Bacc(None, target_bir_lowering=False, debug=False)

    stat_d = nc.dram_tensor("stat", [128, N], BF16, kind="ExternalInput")
    mov_d = nc.dram_tensor("mov", [128, R], BF16, kind="ExternalInput")
    y1_d = nc.dram_tensor("y1", [128, NT, C1], BF16, kind="ExternalInput")
    ftc_d = nc.dram_tensor("ftc", [C, R], F32, kind="ExternalInput")
    out_d = nc.dram_tensor("out", [1, 1], F32, kind="ExternalOutput")

    ident_np = np.eye(128, dtype=NPBF16)
    negbig_np = (np.eye(128) * BIGNEG).astype(NPBF16)
    comb_np = np.zeros((128, C1), dtype=NPBF16)
    for g in range(4):
        for c in range(C1):
            comb_np[32 * g + c, c] = 1.0
    dsel_np = np.zeros((128, 1), dtype=NPBF16)
    for g in range(4):
        dsel_np[32 * g + C, 0] = 1.0
    ident_d = nc.inline_tensor(ident_np, name="identmat")
    negbig_d = nc.inline_tensor(negbig_np, name="negbigmat")
    comb_d = nc.inline_tensor(comb_np, name="combmat")
    dsel_d = nc.inline_tensor(dsel_np, name="denselect")

    from contextlib import ExitStack

    with tile.TileContext(nc) as tc, ExitStack() as ctx:
        consts = ctx.enter_context(tc.tile_pool(name="consts", bufs=1))
        kpool = ctx.enter_context(tc.tile_pool(name="kern", bufs=3))
        epool = ctx.enter_context(tc.tile_pool(name="epi", bufs=2))
        pln = ctx.enter_context(
            tc.tile_pool(name="pln", bufs=2, space=bass.MemorySpace.PSUM)
        )
        pacc = ctx.enter_context(
            tc.tile_pool(name="pacc", bufs=2, space=bass.MemorySpace.PSUM)
        )
        psml = ctx.enter_context(
            tc.tile_pool(name="psml", bufs=1, space=bass.MemorySpace.PSUM)
        )
        ploss_pool = ctx.enter_context(
            tc.tile_pool(name="ploss", bufs=1, space=bass.MemorySpace.PSUM)
        )

        stat_sb = consts.tile([128, N], BF16)
        for cch in range(4):
            sl = slice(cch * (N // 4), (cch + 1) * (N // 4))
            nc.sync.dma_start(out=stat_sb[:, sl], in_=stat_d[:, sl])
        mov_sb = consts.tile([128, R], BF16)
        nc.sync.dma_start(out=mov_sb[:], in_=mov_d[:])
        y1_sb = consts.tile([128, NT, C1], BF16)
        nc.sync.dma_start(out=y1_sb[:], in_=y1_d[:])
        ftc_sb = consts.tile([C, R], F32)
        nc.sync.dma_start(out=ftc_sb[:], in_=ftc_d[:])
        ident_sb = consts.tile([128, 128], BF16)
        nc.sync.dma_start(out=ident_sb[:], in_=ident_d[:])
        negbig_sb = consts.tile([128, 128], BF16)
        nc.sync.dma_start(out=negbig_sb[:], in_=negbig_d[:])
        comb_sb = consts.tile([128, C1], BF16)
        nc.sync.dma_start(out=comb_sb[:], in_=comb_d[:])
        dsel_sb = consts.tile([128, 1], BF16)
        nc.sync.dma_start(out=dsel_sb[:], in_=dsel_d[:])

        ones_1xC = consts.tile([1, C], BF16)
        nc.vector.memset(ones_1xC[:], 1.0)
        ones_Cx1 = consts.tile([C, 1], BF16)
        nc.vector.memset(ones_Cx1[:], 1.0)

        ploss = ploss_pool.tile([1, 512], F32)

        for ic in range(2):
            isl = slice(ic * 512, (ic + 1) * 512)
            psB = pacc.tile([128, 512], F32)
            for pr in range(NT // 2):
                t0 = 2 * pr
                unit = pln.tile([128, 1024], F32)
                for half, (t, base) in enumerate(((t0, 0), (t0 + 1, 64))):
                    nc.tensor.matmul(
                        unit[:, half * 512 : (half + 1) * 512],
                        lhsT=stat_sb[base : base + KROWS, t * 128 : (t + 1) * 128],
                        rhs=mov_sb[base : base + KROWS, isl],
                        start=True,
                        stop=True,
                    )
                for half, t in enumerate((t0, t0 + 1)):
                    if 4 * ic <= t < 4 * ic + 4:
                        off = half * 512 + t * 128 - ic * 512
                        nc.tensor.matmul(
                            unit[:, off : off + 128],
                            lhsT=ident_sb[:],
                            rhs=negbig_sb[:],
                            start=False,
                            stop=True,
                            skip_group_check=True,
                        )
                ksb = kpool.tile([128, 1024], BF16)
                nc.scalar.activation(
                    ksb[:], unit[:], mybir.ActivationFunctionType.Exp
                )
                for half, t in enumerate((t0, t0 + 1)):
                    g = t % 4
                    nc.tensor.matmul(
                        psB[32 * g : 32 * g + C1, :],
                        lhsT=y1_sb[:, t, :],
                        rhs=ksb[:, half * 512 : (half + 1) * 512],
                        start=(t < 4),
                        stop=(t >= NT - 4),
                        skip_group_check=True,
                        tile_position=(0, 32 * g),
                    )

            # epilogue for this i-chunk (all small tiles)
            ky4 = epool.tile([128, 512], BF16, tag="ky4")
            nc.vector.tensor_copy(ky4[0:116, :], psB[0:116, :])
            psc = psml.tile([128, 512], F32, tag="small")
            nc.tensor.matmul(
                psc[0:C1, :],
                lhsT=comb_sb[0:116, :],
                rhs=ky4[0:116, :],
                start=True,
                stop=True,
                skip_group_check=True,
                tile_position=(0, 0),
            )
            nc.tensor.matmul(
                psc[32:33, :],
                lhsT=dsel_sb[0:116, :],
                rhs=ky4[0:116, :],
                start=True,
                stop=True,
                skip_group_check=True,
                tile_position=(0, 32),
            )
            ky = epool.tile([C1, 512], F32, tag="ky")
            nc.vector.tensor_copy(ky[:], psc[0:C1, :])
            dmx = epool.tile([1, 512], F32, tag="dmx")
            nc.vector.tensor_scalar_max(dmx[:], psc[32:33, :], 1e-10)
            rcp = epool.tile([1, 512], F32, tag="rcp")
            nc.vector.reciprocal(rcp[:], dmx[:])
            rcpb = epool.tile([1, 512], BF16, tag="rcpb")
            nc.vector.tensor_copy(rcpb[:], rcp[:])
            psr = psml.tile([128, 512], F32, tag="small")
            nc.tensor.matmul(
                psr[0:C, :],
                lhsT=ones_1xC[:],
                rhs=rcpb[:],
                start=True,
                stop=True,
            )
            ratio = epool.tile([C, 512], F32, tag="ratio")
            nc.vector.tensor_mul(ratio[:], ky[0:C, :], psr[0:C, :])
            dd = epool.tile([C, 512], F32, tag="dd")
            nc.vector.tensor_sub(dd[:], ratio[:], ftc_sb[:, isl])
            dd2 = epool.tile([C, 512], BF16, tag="dd2")
            nc.vector.tensor_mul(dd2[:], dd[:], dd[:])
            nc.tensor.matmul(
                ploss[0:1, :],
                lhsT=ones_Cx1[:],
                rhs=dd2[:],
                start=(ic == 0),
                stop=(ic == 1),
                skip_group_check=True,
            )

        part = epool.tile([1, 1], F32, tag="part")
        nc.vector.reduce_sum(part[:], ploss[0:1, :], axis=mybir.AxisListType.X)
        nc.sync.dma_start(out=out_d[:], in_=part[:])

    nc.compile()
    return nc


_NC_CACHE = None


def _get_nc():
    global _NC_CACHE
    if _NC_CACHE is None:
        _NC_CACHE = _build_nc()
    return _NC_CACHE


def _bf_hi_lo(x):
    hi = x.astype(NPBF16)
    lo = (x - hi.astype(np.float32)).astype(NPBF16)
    return hi, lo


def prepare_in_maps(input, target):
    """Host-side preprocessing: softmax/argmax on the strided subsample,
    log-Beta normalizers, bf16 hi/lo operand splits, and the per-core
    j-rotated layouts."""
    x = np.asarray(input)[:, :, ::DF, ::DF].astype(np.float32)
    t = np.asarray(target)[:, :, ::DF, ::DF]

    m = x.max(axis=1, keepdims=True)
    e = np.exp(x - m)
    probs = e / e.sum(axis=1, keepdims=True)
    f = probs.transpose(0, 2, 3, 1).reshape(-1, C).astype(np.float32)
    y = t.argmax(axis=1).reshape(-1)

    alphas = f / BW + np.float32(1.0)
    b = alphas - np.float32(1.0)
    logf = np.log(f)
    lb = (
        gammaln(alphas.astype(np.float64)).sum(axis=1)
        - gammaln(alphas.sum(axis=1, dtype=np.float64))
    ).astype(np.float32)
    nlb = -lb

    b_hi, b_lo = _bf_hi_lo(b)
    a_hi, a_lo = _bf_hi_lo(logf)
    nlb_hi, nlb_lo = _bf_hi_lo(nlb)

    in_maps = []
    for k in range(NCORES):
        perm = (np.arange(N) + k * R) % N
        stat = np.zeros((128, N), dtype=NPBF16)
        stat[0:19] = b_hi[perm].T
        stat[19:38] = b_hi[perm].T
        stat[38:57] = b_lo[perm].T
        stat[57] = nlb_hi[perm]
        stat[58] = nlb_lo[perm]
        stat[64 : 64 + KROWS] = stat[0:KROWS]

        rows = slice(k * R, (k + 1) * R)
        mov = np.zeros((128, R), dtype=NPBF16)
        mov[0:19] = a_hi[rows].T
        mov[19:38] = a_lo[rows].T
        mov[38:57] = a_hi[rows].T
        mov[57] = NPBF16(1.0)
        mov[58] = NPBF16(1.0)
        mov[64 : 64 + KROWS] = mov[0:KROWS]

        yp = y[perm].reshape(NT, 128)  # [t, p]
        y1 = np.zeros((128, NT, C1), dtype=NPBF16)
        onehot = (yp[:, :, None] == np.arange(C)[None, None, :]).astype(NPBF16)
        y1[:, :, 0:C] = onehot.transpose(1, 0, 2)
        y1[:, :, C] = NPBF16(1.0)

        ftc = np.ascontiguousarray(f[rows].T)

        in_maps.append({"stat": stat, "mov": mov, "y1": y1, "ftc": ftc})
    return in_maps


def run_device(in_maps, trace=False, trace_cores=None):
    nc = _get_nc()
    return run_bass_kernel_spmd(
        nc,
        in_maps,
        core_ids=list(range(NCORES)),
        trace=trace,
        trace_cores=trace_cores,
    )


def kernel(input, target):
    in_maps = prepare_in_maps(input, target)
    res = run_device(in_maps)
    total = np.float32(0.0)
    for r in res.results:
        total += r["out"][0, 0]
    return np.float32(total / np.float32(N))
